# revision 6
# baseline (speedup 1.0000x reference)
"""CrossAttention (B=4, N=M=2048, C=1024, H=16, D=64) on 8 TRN2 cores, v2.

Sharding: core = 2*b + g  (b = batch 0..3, g = head-half 0..1, 8 heads each).

v2 changes vs baseline:
  - KPAD: score matmuls contract over 128 partitions (k zero-padded per
    head) instead of 64.  HW streams bf16 moving data at 2 rows/cycle
    only(?) for K=128; K=64 measured ~3x slower per row on HW.
  - FOLD: k/q projections for head-pairs j>=1 and the output projection
    are emitted as a work queue drained in PE slack inside the attention
    pipeline, instead of serial phases.
  - prologue: v-proj (m-blocked DMA), k-proj j=0, q-proj (j=0, nq=0).
"""

from contextlib import ExitStack

import ml_dtypes
import numpy as np

import concourse.bass as bass
import concourse.mybir as mybir
import concourse.tile as tile
from concourse import bacc, library_config
from concourse.bass_utils import run_bass_kernel_spmd
from concourse.masks import make_identity

dt = mybir.dt
AF = mybir.ActivationFunctionType

B, N, M, C, H = 4, 2048, 2048, 1024, 16
D = C // H            # 64
SCALE = D ** -0.5
CL = C // 2           # 512 channels per core (8 heads)
HL = H // 2           # 8 local heads
P = 128
CT = C // P           # 8 input-channel tiles
DT = CL // P          # 4 local-channel tiles (head pairs)
MT = M // P           # 16 key tiles
NCH = 512             # psum bank width in fp32
NQ = N // NCH         # 4 query chunks
EXPW = 1024           # exp width (2 psum banks)
VA = D + 1            # 65: v columns + ones column
WH = (CT // 2) * CL   # 2048: half of a pre-tiled weight image

F32 = dt.float32
BF16 = dt.bfloat16

KPAD = True           # zero-pad score contraction to K=128
FOLD = True           # fold j>=1 projections + out-proj into the pipeline
AV_FLIP = True        # AV as out[q,65] matmuls (lhsT=pt slice, rhs=va);
                      # needs PE transpose of x back to channel-major


def build_program(reps: int = 1, mode: str = "") -> bass.Bass:
    kpad = KPAD and "nokpad" not in mode
    fold = FOLD and "nofold" not in mode
    av_flip = AV_FLIP and "noflip" not in mode

    nc = bacc.Bacc()
    nc.gpsimd.load_library(library_config.attn)

    qTin = nc.declare_dram_parameter("qTin", [C, N], BF16, isOutput=False)
    kTin = nc.declare_dram_parameter("kTin", [C, M], BF16, isOutput=False)
    vTin = nc.declare_dram_parameter("vTin", [C, M], BF16, isOutput=False)
    wq = nc.declare_dram_parameter("wq", [P, CT * CL], BF16, isOutput=False)
    wk = nc.declare_dram_parameter("wk", [P, CT * CL], BF16, isOutput=False)
    wv = nc.declare_dram_parameter("wv", [P, CT * CL], BF16, isOutput=False)
    wp = nc.declare_dram_parameter("wp", [P, DT * C], BF16, isOutput=False)
    bp = nc.declare_dram_parameter("bp", [P, CT], F32, isOutput=False)
    out = nc.declare_dram_parameter("out", [C, N], F32, isOutput=True)

    with tile.TileContext(nc) as tc, ExitStack() as ctx:
        const_pool = ctx.enter_context(tc.tile_pool(name="consts", bufs=1))
        bp_sb = const_pool.tile([P, CT], F32)
        qT_sb = const_pool.tile([P, DT * N], BF16)
        # k per head, zero-padded to 128 contraction rows when kpad:
        #   kp_sb[hh] rows hh*64..hh*64+64 hold k of head 2j+hh, rest zero.
        if kpad:
            kp0_sb = const_pool.tile([P, DT * M], BF16)
            kp1_sb = const_pool.tile([P, DT * M], BF16)
            kT_of = {0: kp0_sb, 1: kp1_sb}
        else:
            kT_sb = const_pool.tile([P, DT * M], BF16)
        va_sb = const_pool.tile([P, (MT // 2) * HL * 2 * VA], BF16)
        xT_sb = const_pool.tile([P, DT * N], BF16)
        wp_sb = const_pool.tile([P, DT * C], BF16)

        if av_flip:
            ident = const_pool.tile([P, P], BF16)
            make_identity(nc, ident[:])

        va4 = va_sb[:].rearrange("p (r h t e) -> p r h t e", h=HL, t=2, e=VA)

        ps_pool = ctx.enter_context(tc.tile_pool(name="ps", space="PSUM",
                                                 bufs=2))
        pt_pool = ctx.enter_context(tc.tile_pool(name="pt", bufs=3))
        sm_pool = ctx.enter_context(tc.tile_pool(name="sm", bufs=2))
        ob_pool = ctx.enter_context(tc.tile_pool(name="ob", bufs=2))
        # input c-tiles are loaded as column-halves [P, 1024] so compute can
        # start after half the bytes land; k/q halves stay live while their
        # folded projections drain (peak ~32 live + v transit).
        in_pool = ctx.enter_context(tc.tile_pool(name="inT", bufs=40))
        # wk/wq halves stay live into the pipeline (folded projections);
        # wv frees after v-proj.  All six halves must coexist.
        w_pool = ctx.enter_context(tc.tile_pool(name="wcur", bufs=6))

        # One-time init (NOT per rep: a whole-tensor memset is a write
        # barrier against every reader of the previous rep, serializing the
        # repeated body).  The k-proj copies only touch data rows and the
        # va copies only touch [:, :, :D], so pad zeros / the ones column
        # survive across reps.
        if kpad:
            nc.vector.memset(kp0_sb[:], 0.0)
            nc.vector.memset(kp1_sb[:], 0.0)
        nc.vector.memset(va_sb[:], 1.0)  # ones column default
        nc.sync.dma_start(out=bp_sb[:], in_=bp[:, :])
        nc.sync.dma_start(out=wp_sb[:], in_=wp[:, :])

        for _rep in range(reps):
          if True:

            def load_w_half(w_dram, half):
                t = w_pool.tile([P, WH], BF16, tag="w", name="wh")
                nc.sync.dma_start(out=t[:], in_=w_dram[:, half * WH:(half + 1) * WH])
                return t

            def load_ctile(src, ct):
                halves = []
                for h2 in range(2):
                    t = in_pool.tile([P, EXPW], BF16, tag="inT",
                                     name=f"i{ct}_{h2}")
                    nc.sync.dma_start(
                        out=t[:],
                        in_=src[ct * P:(ct + 1) * P,
                                h2 * EXPW:(h2 + 1) * EXPW])
                    halves.append(t)
                return halves

            # ---------- v projection (prologue) ----------
            wvh = [load_w_half(wv, 0), load_w_half(wv, 1)]
            vtiles = [load_ctile(vTin, ct) for ct in range(CT)]
            for mtp in range(MT // 2):
                acc = ps_pool.tile([P, EXPW], F32, tag="big", bufs=2,
                                   name="prv")
                for t in range(2):
                    mt = 2 * mtp + t
                    for qn in range(4):
                        for ct in range(CT):
                            nc.tensor.matmul(
                                acc[:, t * CL + qn * P: t * CL + (qn + 1) * P],
                                vtiles[ct][mt // 8][:, (mt % 8) * P:
                                                     (mt % 8 + 1) * P],
                                wvh[ct // 4][:, (ct % 4) * CL + qn * P:
                                             (ct % 4) * CL + (qn + 1) * P],
                                start=(qn == 0 and ct == 0),
                                stop=(qn == 3 and ct == CT - 1),
                                skip_group_check=True,
                            )
                for t in range(2):
                    blk = va4[:, mtp, :, t, :]       # (P, HL, VA)
                    nc.vector.tensor_copy(
                        blk[:, :, :D],
                        acc[:, t * CL:(t + 1) * CL].rearrange(
                            "p (h d) -> p h d", d=D),
                    )

            # ---------- k / q projection emission helpers ----------
            kw = {"k": [load_w_half(wk, 0), load_w_half(wk, 1)]}
            ktiles = [load_ctile(kTin, ct) for ct in range(CT)]
            qw = {}
            qtiles = []

            def ensure_q_loaded():
                if not qtiles:
                    qw["q"] = [load_w_half(wq, 0), load_w_half(wq, 1)]
                    qtiles.extend(load_ctile(qTin, ct) for ct in range(CT))



            # Folded projection accs share the [P, NCH] "po" psum ring with
            # the output projection (NOT the sc "big" ring: an acc held
            # across queue pops while sc tiles rotate the same ring would
            # deadlock the in-order PE stream).  Quantum = one matmul into a
            # half-width acc; at ct==7 the PSUM->SBUF copy is emitted.
            proj_state = {}

            def proj_mm(which, j, h2, nn2, qn, ct):
                if which == "q":
                    ensure_q_loaded()
                    whs, tiles, src_n = qw["q"], qtiles, N
                else:
                    whs, tiles, src_n = kw["k"], ktiles, M
                key = (which, j, h2, nn2)
                if key not in proj_state:
                    proj_state[key] = ps_pool.tile(
                        [P, NCH], F32, tag="po", bufs=2, name="prj")
                acc = proj_state[key]
                nc.tensor.matmul(
                    acc[:, qn * P:(qn + 1) * P],
                    whs[ct // 4][:, (ct % 4) * CL + j * P:
                                 (ct % 4) * CL + (j + 1) * P],
                    tiles[ct][h2][:, nn2 * NCH + qn * P:
                                  nn2 * NCH + (qn + 1) * P],
                    start=(qn == 0 and ct == 0),
                    stop=(qn == 3 and ct == CT - 1),
                    skip_group_check=True,
                )
                if qn == 3 and ct == CT - 1:
                    del proj_state[key]
                    s0 = j * src_n + h2 * EXPW + nn2 * NCH
                    seg = slice(s0, s0 + NCH)
                    if which == "q":
                        nc.vector.tensor_copy(qT_sb[:, seg], acc[:])
                    elif kpad:
                        nc.vector.tensor_copy(kp0_sb[:64, seg], acc[:64, :])
                        nc.vector.tensor_copy(kp1_sb[64:, seg], acc[64:, :])
                    else:
                        nc.vector.tensor_copy(kT_sb[:, seg], acc[:])

            def emit_proj(which, j):
                for h2 in range(2):
                    for nn2 in range(2):
                        for qn in range(4):
                            for ct in range(CT):
                                proj_mm(which, j, h2, nn2, qn, ct)

            # ---------- prologue projections (minimal): k0 first key-half,
            # q0 first query chunk.  The rest is queued with deadlines.
            for nn2 in range(2):
                for qn in range(4):
                    for ct in range(CT):
                        proj_mm("k", 0, 0, nn2, qn, ct)
            for qn in range(4):
                for ct in range(CT):
                    proj_mm("q", 0, 0, 0, qn, ct)

            # ---------- pipeline ----------
            # j-major order: head-pair j's runs all precede j+1, so folded
            # k/q projections for j+1 have a full block (64 iters) of PE
            # slack before first use.  Out-proj for chunk nq unlocks after
            # run (j=DT-1, nq, hh=1).
            runs = [(nq, j, hh) for j in range(DT) for nq in range(NQ)
                    for hh in range(2)]
            IPR = MT // 2
            NG = len(runs) * IPR

            avs_cur = [None]

            def emit_sc(g):
                nq, j, hh = runs[g // IPR]
                mtp = g % IPR
                sc = ps_pool.tile([P, EXPW], F32, tag="big", bufs=2, name="sc")
                # N=128 strips: measured 0.40 ns/row vs 0.50 at N=512.
                for t in range(2):
                    mt = 2 * mtp + t
                    for qn in range(4):
                        qcol = nq * NCH + qn * P
                        if kpad:
                            nc.tensor.matmul(
                                sc[:, t * NCH + qn * P: t * NCH + (qn + 1) * P],
                                kT_of[hh][:, j * M + mt * P:
                                           j * M + (mt + 1) * P],
                                qT_sb[:, j * N + qcol: j * N + qcol + P],
                                start=True, stop=True,
                            )
                        else:
                            roff = hh * D
                            nc.tensor.matmul(
                                sc[:, t * NCH + qn * P: t * NCH + (qn + 1) * P],
                                kT_sb[roff:roff + D,
                                      j * M + mt * P: j * M + (mt + 1) * P],
                                qT_sb[roff:roff + D, j * N + qcol:
                                      j * N + qcol + P],
                                start=True, stop=True,
                            )
                return sc

            def emit_exp(g, sc):
                pt = pt_pool.tile([P, EXPW], BF16, tag="pt", name="pt")
                nc.scalar.activation(pt[:], sc[:], AF.Exp)
                return pt

            def emit_av(g, pt):
                nq, j, hh = runs[g // IPR]
                mtp = g % IPR
                h = 2 * j + hh
                if mtp == 0:
                    # full-bank tile: start_tensor_calc zeroes the whole
                    # 2KB psum region, so all 4 q-slice accumulators must
                    # share one bank-aligned region with a single group.
                    avs_cur[0] = ps_pool.tile([P, NCH], F32, tag="av",
                                              bufs=2, name="avs")
                avs = avs_cur[0]
                if av_flip:
                    av4 = avs[:, :4 * VA].rearrange("p (q e) -> p q e", e=VA)
                    for t in range(2):
                        for qs in range(4):
                            nc.tensor.matmul(
                                av4[:, qs, :],
                                pt[:, t * NCH + qs * P: t * NCH + (qs + 1) * P],
                                va4[:, mtp, h, t, :],
                                start=(mtp == 0 and t == 0 and qs == 0),
                                stop=(mtp == IPR - 1 and t == 1 and qs == 3),
                                skip_group_check=True,
                            )
                else:
                    for t in range(2):
                        nc.tensor.matmul(
                            avs[:VA, :],
                            va4[:, mtp, h, t, :],
                            pt[:, t * NCH:(t + 1) * NCH],
                            start=(mtp == 0 and t == 0),
                            stop=(mtp == IPR - 1 and t == 1),
                        )

            def emit_norm(run):
                nq, j, hh = run
                roff = hh * D
                avs = avs_cur[0]
                if av_flip:
                    # avs: [128 q, 4 slices, 65]; col 64 = denominator.
                    av4 = avs[:, :4 * VA].rearrange("p (q e) -> p q e", e=VA)
                    rc = sm_pool.tile([P, 4], F32, tag="rc", name="rc")
                    nc.vector.reciprocal(rc[:], av4[:, :, D])
                    stage = sm_pool.tile([P, 4 * D], BF16, tag="st", bufs=2,
                                         name="st")
                    st4 = stage[:].rearrange("p (q d) -> p q d", d=D)
                    for qs in range(4):
                        nc.vector.tensor_scalar_mul(
                            st4[:, qs, :], av4[:, qs, :D], rc[:, qs:qs + 1])
                    # transpose x [128q, 64d] -> [64d, 128q] and store to xT;
                    # queued so the shared "po" psum ring stays FIFO-safe.
                    for qs in range(4):
                        def tr(jj=j, hhh=hh, nnq=nq, q_s=qs, st=stage):
                            s4 = st[:].rearrange("p (q d) -> p q d", d=D)
                            pst = ps_pool.tile([P, NCH], BF16, tag="po",
                                               bufs=2, name="tr")
                            nc.tensor.transpose(pst[:D, :P], s4[:, q_s, :],
                                                ident[:])
                            col = jj * N + nnq * NCH + q_s * P
                            # gpsimd cannot read PSUM on HW; DVE does this.
                            nc.vector.tensor_copy(
                                xT_sb[hhh * D:hhh * D + D, col:col + P],
                                pst[:D, :P])
                        pe_q.append((60, None, tr))
                else:
                    rc = sm_pool.tile([1, NCH], F32, tag="rc", name="rc")
                    nc.vector.reciprocal(rc[:], avs[D:VA, :])
                    bc = sm_pool.tile([D, NCH], F32, tag="bc", name="bc")
                    nc.gpsimd.partition_broadcast(bc[:], rc[:])
                    nc.vector.tensor_mul(
                        xT_sb[roff:roff + D,
                              j * N + nq * NCH: j * N + (nq + 1) * NCH],
                        avs[:D, :],
                        bc[:],
                    )

            # ---------- PE work queue (folded projections + out-proj) ----
            # items: (cost_ns, deadline_iter | None, closure); kept sorted
            # by deadline at assembly, popped strictly FIFO.
            pe_q = []

            def queue_proj(which, j, skip_first=False):
                for h2 in range(2):
                    for nn2 in range(2):
                        if skip_first and h2 == 0 and (nn2 == 0 or which == "k"):
                            continue
                        if which == "k":
                            dl = j * 64 + 4 * h2
                        else:
                            dl = j * 64 + (h2 * 2 + nn2) * 16
                        for qn in range(4):
                            for ct in range(CT):
                                pe_q.append(
                                    (54, dl, lambda w=which, jj=j, hh2=h2,
                                     n2=nn2, q=qn, c=ct:
                                     proj_mm(w, jj, hh2, n2, q, c)))

            po_state = {"tile": None, "left": 0}

            def po_mm(nq, mt8, qn, ct):
                if po_state["left"] == 0:
                    po_state["tile"] = ps_pool.tile([P, NCH], F32, tag="po",
                                                    bufs=2, name="po")
                    po_state["left"] = 4 * DT
                po = po_state["tile"]
                nc.tensor.matmul(
                    po[:, qn * P:(qn + 1) * P],
                    wp_sb[:, ct * C + mt8 * P: ct * C + (mt8 + 1) * P],
                    xT_sb[:, ct * N + nq * NCH + qn * P:
                          ct * N + nq * NCH + (qn + 1) * P],
                    start=(qn == 0 and ct == 0),
                    stop=(qn == 3 and ct == DT - 1),
                    skip_group_check=True,
                )
                po_state["left"] -= 1
                if po_state["left"] == 0:
                    ob = ob_pool.tile([P, NCH], F32, tag="ob", name="ob")
                    nc.vector.tensor_scalar_add(ob[:, :], po[:, :],
                                                bp_sb[:, mt8:mt8 + 1])
                    if "X" not in mode:
                        nc.sync.dma_start(
                            out=out[mt8 * P:(mt8 + 1) * P,
                                    nq * NCH:(nq + 1) * NCH],
                            in_=ob[:])

            def queue_po(nq):
                for mt8 in range(CT):
                    for qn in range(4):
                        for ct in range(DT):
                            pe_q.append((54, None, lambda n=nq, m=mt8, q=qn,
                                         c=ct: po_mm(n, m, q, c)))

            if fold:
                queue_proj("k", 0, skip_first=True)
                queue_proj("q", 0, skip_first=True)
                for j in range(1, DT):
                    queue_proj("k", j)
                    queue_proj("q", j)
                pe_q.sort(key=lambda it: it[1] if it[1] is not None else 1 << 30)
            else:
                for h2 in range(2):
                    for nn2 in range(2):
                        if h2 == 0 and nn2 == 0:
                            continue
                        for qn in range(4):
                            for ct in range(CT):
                                proj_mm("q", 0, h2, nn2, qn, ct)
                for nn2 in range(2):
                    for qn in range(4):
                        for ct in range(CT):
                            proj_mm("k", 0, 1, nn2, qn, ct)
                for j in range(1, DT):
                    emit_proj("k", j)
                    emit_proj("q", j)

            # PE slack per iteration at ACT pace (~1040ns):
            # kpad: 2 sc ~250 + 2 av ~250 -> ~540ns slack
            # else: 2 sc ~430 + 2 av ~430 -> ~180ns slack
            # Stretch the fold drain across all iterations: ~78us of queued
            # work / 256 iters ~ 330ns.  Draining faster empties the queue
            # by ~iter 150, after which the PE idles ~400ns/iter behind the
            # ACT pace and drops out of its top p-state.
            budget_per_iter = 330 if kpad else 180
            debt = [0.0]

            def drain_queue():
                debt[0] += budget_per_iter
                while pe_q and pe_q[0][0] <= debt[0]:
                    cost, _nk, fn = pe_q.pop(0)
                    debt[0] -= cost
                    fn()
                if not pe_q:
                    debt[0] = 0.0

            def force_drain_for(g_next):
                """FIFO-pop every item whose deadline is <= g_next.  The
                queue is deadline-sorted (appended po/tr items have None),
                so this is a prefix pop; a run's sc matmuls must come after
                its k/q projection writes in the in-order PE stream."""
                while pe_q and pe_q[0][1] is not None and pe_q[0][1] <= g_next:
                    _cost, _dl, fn = pe_q.pop(0)
                    fn()

            # Emission order per iteration: sc(g+1) -> exp(g) [ACT] ->
            # fold-pops -> av(g).  The fold matmuls sit between sc(g+1) and
            # av(g) in the in-order PE stream, so the PE stays busy while
            # the ACT computes exp(g) that av(g) depends on.
            sc_tiles = {0: emit_sc(0)}
            for g in range(NG):
                if g + 1 < NG:
                    force_drain_for(g + 1)
                    sc_tiles[g + 1] = emit_sc(g + 1)
                pt = emit_exp(g, sc_tiles.pop(g))
                drain_queue()
                emit_av(g, pt)
                if g % IPR == IPR - 1:
                    r = g // IPR
                    emit_norm(runs[r])
                    nq_r, j_r, hh_r = runs[r]
                    if j_r == DT - 1 and hh_r == 1:
                        queue_po(nq_r)
            while pe_q:
                _cost, _nk, fn = pe_q.pop(0)
                fn()

    nc.compile()
    return nc


_NC_CACHE = {}


def _get_program(reps: int = 1, mode: str = ""):
    key = (reps, mode)
    if key not in _NC_CACHE:
        _NC_CACHE[key] = build_program(reps, mode)
    return _NC_CACHE[key]


def _tile_w(wT_slice):
    c, cl = wT_slice.shape
    return np.ascontiguousarray(
        wT_slice.reshape(c // P, P, cl).transpose(1, 0, 2).reshape(P, -1))


def make_in_maps(query, key, value, Wq, Wk, Wv, Wp, bp):
    query = np.asarray(query, dtype=np.float32)
    key = np.asarray(key, dtype=np.float32)
    value = np.asarray(value, dtype=np.float32)
    Wq = np.asarray(Wq, dtype=np.float32)
    Wk = np.asarray(Wk, dtype=np.float32)
    Wv = np.asarray(Wv, dtype=np.float32)
    Wp = np.asarray(Wp, dtype=np.float32)
    bp = np.asarray(bp, dtype=np.float32)

    wqT = np.ascontiguousarray(Wq.T) * np.float32(SCALE)
    wkT = np.ascontiguousarray(Wk.T)
    wvT = np.ascontiguousarray(Wv.T)
    wpT = np.ascontiguousarray(Wp.T)
    zeros_bp = np.zeros_like(bp)

    in_maps = []
    for core in range(8):
        b, g = divmod(core, 2)
        sl = slice(g * CL, (g + 1) * CL)
        bpc = (bp if g == 0 else zeros_bp)
        in_maps.append({
            "qTin": np.ascontiguousarray(query[b].T).astype(ml_dtypes.bfloat16),
            "kTin": np.ascontiguousarray(key[b].T).astype(ml_dtypes.bfloat16),
            "vTin": np.ascontiguousarray(value[b].T).astype(ml_dtypes.bfloat16),
            "wq": _tile_w(wqT[:, sl]).astype(ml_dtypes.bfloat16),
            "wk": _tile_w(wkT[:, sl]).astype(ml_dtypes.bfloat16),
            "wv": _tile_w(wvT[:, sl]).astype(ml_dtypes.bfloat16),
            "wp": _tile_w(wpT[sl, :]).astype(ml_dtypes.bfloat16),
            "bp": np.ascontiguousarray(bpc.reshape(CT, P).T),
        })
    return in_maps


def combine_outputs(results):
    out = np.empty((B, N, C), dtype=np.float32)
    for b in range(B):
        part = results[2 * b]["out"] + results[2 * b + 1]["out"]
        out[b] = part.T
    return out


def kernel(**inputs) -> np.ndarray:
    nc = _get_program()
    in_maps = make_in_maps(**inputs)
    res = run_bass_kernel_spmd(nc, in_maps, list(range(8)))
    return combine_outputs(res.results)


if __name__ == "__main__":
    nc = _get_program()
    print("program built ok")
